# revision 8
# baseline (speedup 1.0000x reference)
"""Distributed Trainium2 kernel for causal softcap attention (dense transformer).

Problem: B=4, T=2048, C=2048, H=16 heads of D=128.
  qkv = x @ w_qkv; rope(q), rope(k); s = q k^T / sqrt(D);
  s = 50*tanh(s/50); causal mask; p = softmax(s); o = p v; out = o @ w_out

Sharding (8 cores): core i = (batch b = i//2, head-group g = i%2 of 8 heads).
  - QKV: x^T kept in SBUF bf16 (half the sequence at a time); Q^T,K^T via
    W-stationary matmuls in [d,t] layout (RoPE + scores need no transposes);
    V via x-stationary matmuls in natural [t,d] layout for the PV matmul.
  - Scores computed transposed, S^T[k,q]; softmax denominator formed by
    DVE-accumulating exp tiles and reducing over partitions with a
    ones-matmul that also broadcasts sums to all 128 partitions.
  - Softcap+softmax: one Tanh + one Exp ACT pass per [128,1024] score pair.
    No max-subtraction needed: |softcapped| <= 50 keeps exp in fp32 range.
  - Per q-tile (512 rows) the 8 heads' normalized O^T are AllGathered
    between the core pair -> full 16-head o^T for that q-tile.
  - Out-projection split by OUTPUT COLUMNS: each core holds its 1024-column
    half of w_out and computes all 2048 rows for it (identical SPMD graph,
    per-core data). Host concatenates halves.
"""

import sys

sys.path.insert(0, "/opt/trn_rl_repo")

import numpy as np
import ml_dtypes

import concourse.bass as bass
import concourse.mybir as mybir
import concourse.tile as tile
from concourse import bacc

F32 = mybir.dt.float32
F32R = mybir.dt.float32r
BF16 = mybir.dt.bfloat16
AF = mybir.ActivationFunctionType

B, T, C, H = 4, 2048, 2048, 16
D = 128
HL = 8          # local heads per core
QT = 512        # q tile (free dim)
NQT = T // QT   # 4
NCC = C // 128  # 16 contraction chunks
SOFTCAP = 50.0
MAX_WAVELENGTH = 10000
N_CORES = 8


def _rope_tables():
    frac = 2.0 * np.arange(64) / 128.0
    timescale = MAX_WAVELENGTH ** frac
    pos = np.arange(T)[:, None]
    inp = pos / timescale[None, :]               # [T,64]
    cos = np.cos(inp).T.astype(np.float32)       # [64,T]
    sin = np.sin(inp).T.astype(np.float32)
    cosD = np.concatenate([cos, cos], 0).astype(ml_dtypes.bfloat16)
    sinD = np.concatenate([sin, sin], 0).astype(ml_dtypes.bfloat16)
    return np.ascontiguousarray(cosD), np.ascontiguousarray(sinD)


def _mask_cat():
    # maskcat[k, j*512+q] = 1 if q >= j*128 + k  (j = diagonal tile index)
    k = np.arange(128)[:, None]
    q = np.arange(QT)[None, :]
    tiles = [(q >= j * 128 + k).astype(np.float32) for j in range(4)]
    return np.concatenate(tiles, axis=1).astype(ml_dtypes.bfloat16)  # [128,2048]


def build_nc(sim_single=False):
    nc = bacc.Bacc("TRN2", target_bir_lowering=False, debug=False,
                   num_devices=1 if sim_single else N_CORES)

    xT = nc.dram_tensor("xT", [C, T], BF16, kind="ExternalInput")
    wq = nc.dram_tensor("wq", [C, HL * D], BF16, kind="ExternalInput")
    wk = nc.dram_tensor("wk", [C, HL * D], BF16, kind="ExternalInput")
    wv = nc.dram_tensor("wv", [C, HL * D], BF16, kind="ExternalInput")
    wo = nc.dram_tensor("wo", [C, 1024], BF16, kind="ExternalInput")
    out = nc.dram_tensor("out", [T, 1024], F32, kind="ExternalOutput")

    # DRAM views chunked to 128-partition tiles
    xT_d = xT.ap().rearrange("(cc p) t -> p cc t", p=128)
    wq_d = wq.ap().rearrange("(cc p) f -> p cc f", p=128)
    wk_d = wk.ap().rearrange("(cc p) f -> p cc f", p=128)
    wv_d = wv.ap().rearrange("(cc p) f -> p cc f", p=128)
    wo_d = wo.ap().rearrange("(cc p) e -> p cc e", p=128)

    cosD_np, sinD_np = _rope_tables()
    mask_np = _mask_cat()
    c1 = float(1.0 / (np.sqrt(float(D)) * SOFTCAP))

    with tile.TileContext(nc) as tc:
        with (
            tc.tile_pool(name="persist", bufs=1) as persist,
            tc.tile_pool(name="psum", bufs=1, space="PSUM") as psp,
            tc.tile_pool(name="dram", bufs=1, space="DRAM") as dram,
        ):
            q_sb = [persist.tile([128, T], BF16, name=f"q{h}", tag=f"q{h}")
                    for h in range(HL)]
            k_sb = [persist.tile([128, T], BF16, name=f"k{h}", tag=f"k{h}")
                    for h in range(HL)]
            # v_sb[vg]: [128 t, 16 tc, 512 f] -> natural V, heads 4vg..4vg+3
            v_sb = [persist.tile([128, NCC, 512], BF16, name=f"v{vg}", tag=f"v{vg}")
                    for vg in range(2)]

            # ---- phase 1: QKV projection + rope (two sequence halves) ----
            with (
                tc.tile_pool(name="xpool", bufs=1) as xpool,
                tc.tile_pool(name="ropetmp", bufs=2) as ropetmp,
            ):
                cosT = xpool.tile([128, T], BF16, name="cosT", tag="cosT")
                sinT = xpool.tile([128, T], BF16, name="sinT", tag="sinT")
                nc.sync.dma_start(out=cosT[:], in_=nc.inline_tensor(cosD_np, name="cos_c").ap())
                nc.sync.dma_start(out=sinT[:], in_=nc.inline_tensor(sinD_np, name="sin_c").ap())

                def rope_store(ps, dst, tb):
                    tsl = bass.ts(tb, QT)
                    ta = ropetmp.tile([128, QT], F32, tag="ra")
                    tb2 = ropetmp.tile([128, QT], F32, tag="rb")
                    nc.vector.tensor_mul(ta[0:64, :], ps[0:64, :], cosT[0:64, tsl])
                    nc.vector.tensor_mul(tb2[0:64, :], ps[64:128, :], sinT[0:64, tsl])
                    nc.vector.tensor_sub(dst[0:64, tsl], ta[0:64, :], tb2[0:64, :])
                    nc.vector.tensor_mul(ta[64:128, :], ps[64:128, :], cosT[64:128, tsl])
                    nc.vector.tensor_mul(tb2[64:128, :], ps[0:64, :], sinT[64:128, tsl])
                    nc.vector.tensor_add(dst[64:128, tsl], ta[64:128, :], tb2[64:128, :])

                for th in range(2):          # sequence half
                    xh = xpool.tile([128, NCC, T // 2], BF16, tag="xh", bufs=1)
                    nc.sync.dma_start(
                        out=xh[:], in_=xT_d[:, :, th * (T // 2):(th + 1) * (T // 2)]
                    )
                    for h in range(HL):
                        hsl = bass.ts(h, D)
                        wqh = xpool.tile([128, NCC, D], BF16, tag="wst", bufs=2)
                        nc.sync.dma_start(out=wqh[:], in_=wq_d[:, :, hsl])
                        wkh = xpool.tile([128, NCC, D], BF16, tag="wst", bufs=2)
                        nc.sync.dma_start(out=wkh[:], in_=wk_d[:, :, hsl])
                        for tbl in range(2):  # t block within half
                            tb = 2 * th + tbl
                            lsl = bass.ts(tbl, QT)
                            psq = psp.tile([128, QT], F32, tag="pk", bufs=2)
                            for cc in range(NCC):
                                nc.tensor.matmul(
                                    psq[:], wqh[:, cc, :], xh[:, cc, lsl],
                                    start=(cc == 0), stop=(cc == NCC - 1),
                                )
                            rope_store(psq, q_sb[h], tb)
                            psk = psp.tile([128, QT], F32, tag="pk", bufs=2)
                            for cc in range(NCC):
                                nc.tensor.matmul(
                                    psk[:], wkh[:, cc, :], xh[:, cc, lsl],
                                    start=(cc == 0), stop=(cc == NCC - 1),
                                )
                            rope_store(psk, k_sb[h], tb)
                    for vg in range(2):
                        wvh = xpool.tile([128, NCC, 512], BF16, tag="wst", bufs=2)
                        nc.sync.dma_start(out=wvh[:], in_=wv_d[:, :, bass.ts(vg, 512)])
                        for tl in range(8):   # t chunk of 128 within half
                            tcc = 8 * th + tl
                            psv = psp.tile([128, 512], F32, tag="pk", bufs=2)
                            for cc in range(NCC):
                                nc.tensor.matmul(
                                    psv[:], xh[:, cc, bass.ts(tl, 128)],
                                    wvh[:, cc, :],
                                    start=(cc == 0), stop=(cc == NCC - 1),
                                )
                            nc.vector.tensor_copy(v_sb[vg][:, tcc, :], psv[:])

            # ---- phase 2: attention + AllGather + out-projection ----
            with tc.tile_pool(name="att", bufs=1) as att:
                wo_sb = att.tile([128, NCC, 1024], BF16, name="wo_sb", tag="wo_sb")
                nc.sync.dma_start(out=wo_sb[:], in_=wo_d)
                maskc = att.tile([128, 4 * QT], BF16, name="maskc", tag="maskc")
                nc.sync.dma_start(out=maskc[:], in_=nc.inline_tensor(mask_np, name="maskc_c").ap())
                ones_r = att.tile([128, 128], F32R, name="ones_r", tag="ones_r")
                nc.sync.dma_start(
                    out=ones_r[:],
                    in_=nc.inline_tensor(np.ones((128, 128), np.float32), name="ones_c").ap().bitcast(F32R),
                )

                for qt in range(NQT):
                    qsl = bass.ts(qt, QT)
                    ag_in = dram.tile([HL * D, QT], BF16, name=f"ag_in{qt}", tag=f"ag_in{qt}")
                    ag_out = dram.tile([2 * HL * D, QT], BF16, name=f"ag_out{qt}", tag=f"ag_out{qt}")
                    npairs = 2 * (qt + 1)
                    for h in range(HL):
                        vg, fcol = h // 4, (h % 4) * 128
                        o_ps = psp.tile([128, QT], F32, tag="o", bufs=2)
                        sumacc = att.tile([128, 2 * QT], F32R, tag="sumacc", bufs=2)
                        for p in range(npairs):
                            s_ps = psp.tile([128, 2 * QT], F32, tag="s", bufs=2)
                            for half in range(2):
                                kt = 2 * p + half
                                nc.tensor.matmul(
                                    s_ps[:, bass.ts(half, QT)],
                                    k_sb[h][:, bass.ts(kt, 128)],
                                    q_sb[h][:, qsl],
                                    start=True, stop=True,
                                )
                            t_sb = att.tile([128, 2 * QT], F32, tag="tanh", bufs=2)
                            nc.scalar.activation(t_sb[:], s_ps[:], AF.Tanh, scale=c1)
                            e_sb = att.tile([128, 2 * QT], BF16, tag="e", bufs=3)
                            nc.scalar.activation(e_sb[:], t_sb[:], AF.Exp,
                                                 scale=SOFTCAP)
                            j0 = 2 * p - 4 * qt
                            if j0 >= 0:
                                nc.vector.tensor_mul(
                                    e_sb[:], e_sb[:],
                                    maskc[:, j0 * QT:(j0 + 2) * QT],
                                )
                            if p == 0:
                                nc.vector.tensor_copy(sumacc[:], e_sb[:])
                            else:
                                nc.vector.tensor_add(sumacc[:], sumacc[:], e_sb[:])
                            for half in range(2):
                                kt = 2 * p + half
                                nc.tensor.matmul(
                                    o_ps[:],
                                    v_sb[vg][:, kt, fcol:fcol + 128],
                                    e_sb[:, bass.ts(half, QT)],
                                    start=(p == 0 and half == 0),
                                    stop=(p == npairs - 1 and half == 1),
                                )
                        srep = psp.tile([128, 2 * QT], F32, tag="s", bufs=2)
                        for half in range(2):
                            nc.tensor.matmul(srep[:, bass.ts(half, QT)],
                                             ones_r[:], sumacc[:, bass.ts(half, QT)],
                                             start=True, stop=True)
                        tot = att.tile([128, QT], F32, tag="tot", bufs=2)
                        nc.vector.tensor_copy(tot[:], srep[:, 0:QT])
                        nc.vector.tensor_add(tot[:], tot[:], srep[:, QT:2 * QT])
                        recip = att.tile([128, QT], F32, tag="recip", bufs=2)
                        nc.vector.reciprocal(recip[:], tot[:])
                        onorm = att.tile([128, QT], BF16, tag="onorm", bufs=2)
                        nc.vector.tensor_mul(onorm[:], o_ps[:], recip[:])
                        nc.sync.dma_start(out=ag_in[bass.ts(h, D), :], in_=onorm[:])

                    if sim_single:
                        nc.sync.dma_start(out=ag_out[0:HL * D, :], in_=ag_in[:])
                        nc.sync.dma_start(out=ag_out[HL * D:, :], in_=ag_in[:])
                    else:
                        nc.gpsimd.collective_compute(
                            "AllGather",
                            mybir.AluOpType.bypass,
                            replica_groups=[[2 * i, 2 * i + 1] for i in range(4)],
                            ins=[ag_in[:].opt()],
                            outs=[ag_out[:].opt()],
                        )

                    # out-projection for this q tile (this core's 1024 e-cols)
                    of_sb = att.tile([128, NCC, QT], BF16, tag="of", bufs=2)
                    nc.sync.dma_start(
                        out=of_sb[:], in_=ag_out[:].rearrange("(cc p) q -> p cc q", p=128)
                    )
                    for qs in range(4):
                        for ec in range(2):
                            po = psp.tile([128, QT], F32, tag="pk", bufs=2)
                            for cc in range(NCC):
                                nc.tensor.matmul(
                                    po[:],
                                    of_sb[:, cc, bass.ts(qs, 128)],
                                    wo_sb[:, cc, bass.ts(ec, QT)],
                                    start=(cc == 0), stop=(cc == NCC - 1),
                                )
                            ot = att.tile([128, QT], F32, tag="ot", bufs=3)
                            nc.vector.tensor_copy(ot[:], po[:])
                            nc.sync.dma_start(
                                out=out.ap()[qt * QT + qs * 128: qt * QT + (qs + 1) * 128,
                                             bass.ts(ec, QT)],
                                in_=ot[:],
                            )

    nc.compile()
    return nc


_NC_CACHE = None


def _get_nc():
    global _NC_CACHE
    if _NC_CACHE is None:
        _NC_CACHE = build_nc()
    return _NC_CACHE


def make_in_maps(x, w_qkv, w_out):
    bf = ml_dtypes.bfloat16
    x = np.asarray(x, np.float32)
    w_qkv = np.asarray(w_qkv, np.float32)
    w_out = np.asarray(w_out, np.float32)
    wq_all = w_qkv[:, 0 * H * D:1 * H * D]
    wk_all = w_qkv[:, 1 * H * D:2 * H * D]
    wv_all = w_qkv[:, 2 * H * D:3 * H * D]
    in_maps = []
    for i in range(N_CORES):
        b, g = i // 2, i % 2
        hsl = slice(g * HL * D, (g + 1) * HL * D)
        in_maps.append({
            "xT": np.ascontiguousarray(x[b].T).astype(bf),
            "wq": np.ascontiguousarray(wq_all[:, hsl]).astype(bf),
            "wk": np.ascontiguousarray(wk_all[:, hsl]).astype(bf),
            "wv": np.ascontiguousarray(wv_all[:, hsl]).astype(bf),
            "wo": np.ascontiguousarray(w_out[:, g * 1024:(g + 1) * 1024]).astype(bf),
        })
    return in_maps


def assemble(results):
    out = np.empty((B, T, C), np.float32)
    for b in range(B):
        out[b, :, 0:1024] = results[2 * b]["out"]
        out[b, :, 1024:2048] = results[2 * b + 1]["out"]
    return out


def kernel(x, mask, w_qkv, w_out):
    from concourse.bass_utils import run_bass_kernel_spmd

    nc = _get_nc()
    in_maps = make_in_maps(x, w_qkv, w_out)
    res = run_bass_kernel_spmd(nc, in_maps, core_ids=list(range(N_CORES)))
    return assemble(res.results)
